# revision 1
# baseline (speedup 1.0000x reference)
"""Trainium2 Bass kernel: multi-head cross-attention (B=4, Sq=Skv=2048,
query_dim=1024, kv_dim=768, 16 heads x 64).

Sharding: 8 cores = data-parallel over batch (4) x tensor-parallel over
heads (2 groups of 8 heads). Each core computes, for its (batch,
head-group):
    Qt = (Wq_shard.T @ query_b.T) + bq   -> [512, 2048]  (head-major, transposed)
    Kt = (Wk_shard.T @ key_b.T)   + bk   -> [512, 2048]
    V  = (value_b @ Wv_shard)            -> [2048, 512]  (natural, + ones col)
    per head h: St = K_h @ Q_h.T (k-major scores), P = exp(St/8),
                At[d,q] (+ sumexp row via ones col) = V_aug.T @ P
    E = At * (1/sumexp) + bv  (head-major, transposed)
    out_t = Wo_shard.T @ E               -> [1024, 2048]  (partial, transposed)
Host sums the two head-group partials per batch, transposes, adds bo.

All activations are fed to the device pre-transposed by the host so no
on-device transposes are needed anywhere. Softmax needs no max-subtract:
the logits are bounded (~|2.5|) for this problem's data distribution.
"""

from contextlib import ExitStack

import numpy as np

import concourse.bacc as bacc
import concourse.mybir as mybir
import concourse.tile as tile
from concourse.bass_utils import run_bass_kernel_spmd

F32 = mybir.dt.float32
F32R = mybir.dt.float32r
AF = mybir.ActivationFunctionType

B = 4
S = 2048  # both Sq and Skv
FQ = 1024  # query in-dim
FKV = 768  # key/value in-dim
DH = 512  # per-core hidden (8 heads x 64)
NH = 8  # heads per core
D = 64  # head dim
SCALE = 0.125  # 1/sqrt(64)
N_CORES = 8

KC_Q = FQ // 128  # 8
KC_KV = FKV // 128  # 6
MT = DH // 128  # 4
KT = S // 128  # 16
QH = 2  # q halves of 1024
QW = S // QH  # 1024


def _emit_projections(nc, tc, io, persist, qt, kt_, vt, bq_sb, bk_sb):
    xq, xk, xv = io["xq_t"], io["xk_t"], io["xv_t"]
    with ExitStack() as st8:
        projp = st8.enter_context(tc.tile_pool(name="proj", bufs=1))
        wq_sb = [projp.tile([128, DH], F32R, tag=f"wq{i}", name=f"wq{i}") for i in range(KC_Q)]
        wk_sb = [projp.tile([128, DH], F32R, tag=f"wk{i}", name=f"wk{i}") for i in range(KC_KV)]
        wv_sb = [projp.tile([128, DH], F32R, tag=f"wv{i}", name=f"wv{i}") for i in range(KC_KV)]
        for i in range(KC_Q):
            nc.sync.dma_start(out=wq_sb[i], in_=io["wq"][i * 128 : (i + 1) * 128, :])

        # Q and K projections: transposed head-major outputs, accumulated
        # over in-dim chunks; activations streamed in q-halves.
        with tc.tile_pool(name="qkps", bufs=4, space="PSUM") as pps:
            for dst, w_sb, x_d, nkc, bias in (
                (qt, wq_sb, xq, KC_Q, bq_sb),
                (kt_, wk_sb, xk, KC_KV, bk_sb),
            ):
                if dst is kt_:
                    for i in range(KC_KV):
                        nc.sync.dma_start(
                            out=wk_sb[i], in_=io["wk"][i * 128 : (i + 1) * 128, :]
                        )
                for qh in range(QH):
                    ps = [
                        pps.tile([128, QW], F32, tag="pp", name=f"pp{m}")
                        for m in range(MT)
                    ]
                    for kc in range(nkc):
                        xt = projp.tile([128, QW], F32R, tag="x", bufs=5, name="xt")
                        nc.sync.dma_start(
                            out=xt,
                            in_=x_d[kc * 128 : (kc + 1) * 128, qh * QW : (qh + 1) * QW],
                        )
                        for m in range(MT):
                            lhs = w_sb[kc][:, m * 128 : (m + 1) * 128]
                            for qc in range(2):
                                sl = slice(qc * 512, (qc + 1) * 512)
                                nc.tensor.matmul(
                                    ps[m][:, sl],
                                    lhs,
                                    xt[:, sl],
                                    start=(kc == 0),
                                    stop=(kc == nkc - 1),
                                )
                    for m in range(MT):
                        nc.vector.tensor_scalar_add(
                            dst[m][:, qh * QW : (qh + 1) * QW],
                            ps[m],
                            bias[:, m : m + 1],
                        )

        # V projection: natural layout, stationary = xv_t chunk, moving = wv.
        for i in range(KC_KV):
            nc.sync.dma_start(out=wv_sb[i], in_=io["wv"][i * 128 : (i + 1) * 128, :])
        ones_col = projp.tile([128, NH, 1], F32, tag="ones", name="ones_col")
        nc.vector.memset(ones_col, 1.0)
        with tc.tile_pool(name="vps", bufs=8, space="PSUM") as ppsv:
            for vh in range(2):
                psv = [
                    ppsv.tile([128, DH], F32, tag="pv", name=f"pv{i}")
                    for i in range(8)
                ]
                for kc in range(KC_KV):
                    xt = projp.tile([128, QW], F32R, tag="x", bufs=5, name="xt")
                    nc.sync.dma_start(
                        out=xt,
                        in_=xv[kc * 128 : (kc + 1) * 128, vh * QW : (vh + 1) * QW],
                    )
                    for ki in range(8):
                        nc.tensor.matmul(
                            psv[ki],
                            xt[:, ki * 128 : (ki + 1) * 128],
                            wv_sb[kc],
                            start=(kc == 0),
                            stop=(kc == KC_KV - 1),
                        )
                for ki in range(8):
                    ktg = vh * 8 + ki
                    nc.vector.tensor_copy(
                        vt[ktg][:, :, 0:D], psv[ki].rearrange("p (h d) -> p h d", h=NH)
                    )
                    nc.vector.tensor_copy(vt[ktg][:, :, D : D + 1], ones_col)


def _emit(nc, tc, io):
    out_t = io["out_t"]
    with ExitStack() as stk:
        persist = stk.enter_context(tc.tile_pool(name="persist", bufs=1))

        # biases as [128, 4] (column m = bias chunk m; element (p, m) = b[m*128+p])
        bq_sb = persist.tile([128, MT], F32, tag="bq")
        bk_sb = persist.tile([128, MT], F32, tag="bk")
        bv_sb = persist.tile([128, MT], F32, tag="bv")
        nc.sync.dma_start(out=bq_sb, in_=io["bq"].rearrange("(m p) -> p m", p=128))
        nc.sync.dma_start(out=bk_sb, in_=io["bk"].rearrange("(m p) -> p m", p=128))
        nc.sync.dma_start(out=bv_sb, in_=io["bv"].rearrange("(m p) -> p m", p=128))

        qt = [persist.tile([128, S], F32R, tag=f"qt{i}", name=f"qt{i}") for i in range(MT)]
        kt_ = [persist.tile([128, S], F32R, tag=f"kt{i}", name=f"kt{i}") for i in range(MT)]
        # V tiles: [128, 8 heads, 65] -- cols 0:64 data, col 64 = ones (sumexp row)
        vt = [
            persist.tile([128, NH, D + 1], F32R, tag=f"vt{i}", name=f"vt{i}")
            for i in range(KT)
        ]
        et = [persist.tile([128, S], F32R, tag=f"et{i}", name=f"et{i}") for i in range(MT)]

        _emit_projections(nc, tc, io, persist, qt, kt_, vt, bq_sb, bk_sb)

        # ---------------- attention ----------------
        attnp = stk.enter_context(tc.tile_pool(name="attn", bufs=1))
        rdp = stk.enter_context(tc.tile_pool(name="rdp", bufs=2, space="DRAM"))
        wo_sb = [attnp.tile([128, FQ], F32R, tag=f"wo{i}", name=f"wo{i}") for i in range(MT)]
        for i in range(MT):
            nc.sync.dma_start(out=wo_sb[i], in_=io["wo"][i * 128 : (i + 1) * 128, :])

        with tc.tile_pool(name="stps", bufs=2, space="PSUM") as pps_st, tc.tile_pool(
            name="atps", bufs=2, space="PSUM"
        ) as pps_at:
            at_tiles = {}

            def emit_qk_exp(h, qh, kt):
                ht, hr = divmod(h, 2)
                st_ = pps_st.tile([128, QW], F32, tag="st", name="st")
                lhs = kt_[ht][hr * D : (hr + 1) * D, kt * 128 : (kt + 1) * 128]
                for qc in range(2):
                    sl = slice(qc * 512, (qc + 1) * 512)
                    qsl = slice(qh * QW + qc * 512, qh * QW + (qc + 1) * 512)
                    nc.tensor.matmul(
                        st_[:, sl],
                        lhs,
                        qt[ht][hr * D : (hr + 1) * D, qsl],
                        start=True,
                        stop=True,
                    )
                pt = attnp.tile([128, QW], F32R, tag="pt", bufs=4, name="pt")
                nc.scalar.activation(pt, st_, AF.Exp, scale=SCALE)
                return pt

            def emit_norm(h, qh):
                ht, hr = divmod(h, 2)
                at = at_tiles.pop((h, qh))
                r = attnp.tile([1, QW], F32, tag="r", bufs=2, name="r")
                nc.vector.reciprocal(r, at[D : D + 1, :])
                bc = attnp.tile([D, QW], F32, tag="bc", bufs=2, name="bc")
                nc.gpsimd.partition_broadcast(bc, r)
                tmp = attnp.tile([D, QW], F32, tag="tmp", bufs=2, name="tmp")
                nc.vector.tensor_mul(tmp, at[0:D, :], bc)
                nc.vector.tensor_scalar_add(
                    et[ht][hr * D : (hr + 1) * D, qh * QW : (qh + 1) * QW],
                    tmp,
                    bv_sb[hr * D : (hr + 1) * D, ht : ht + 1],
                )

            def emit_pv(h, qh, kt, pt):
                if kt == 0:
                    at_tiles[(h, qh)] = pps_at.tile([D + 1, QW], F32, tag="at", name="at")
                at = at_tiles[(h, qh)]
                vsl = vt[kt][:, h, :]
                for qc in range(2):
                    sl = slice(qc * 512, (qc + 1) * 512)
                    nc.tensor.matmul(
                        at[:, sl],
                        vsl,
                        pt[:, sl],
                        start=(kt == 0),
                        stop=(kt == KT - 1),
                    )
                if kt == KT - 1:
                    emit_norm(h, qh)

            steps = [
                (h, qh, kt) for h in range(NH) for qh in range(QH) for kt in range(KT)
            ]
            pts = {steps[0]: emit_qk_exp(*steps[0])}
            for i, step in enumerate(steps):
                if i + 1 < len(steps):
                    pts[steps[i + 1]] = emit_qk_exp(*steps[i + 1])
                emit_pv(*step, pts.pop(step))

        # ---------------- output projection ----------------
        # out_t[ot] = sum_kc wo[kc][:, ot].T @ E[kc]
        with tc.tile_pool(name="ops", bufs=2, space="PSUM") as pps_o, tc.tile_pool(
            name="osb", bufs=2
        ) as osbp:
            for ot in range(FQ // 128):
                po = pps_o.tile([128, S], F32, tag="po", name="po")
                for kc in range(MT):
                    lhs = wo_sb[kc][:, ot * 128 : (ot + 1) * 128]
                    for qc in range(4):
                        sl = slice(qc * 512, (qc + 1) * 512)
                        nc.tensor.matmul(
                            po[:, sl],
                            lhs,
                            et[kc][:, sl],
                            start=(kc == 0),
                            stop=(kc == MT - 1),
                        )
                ob = osbp.tile([128, S], F32, tag="ob", name="ob")
                nc.scalar.copy(ob, po)
                nc.sync.dma_start(out=out_t[ot * 128 : (ot + 1) * 128, :], in_=ob)


_CACHED = {}


def _build():
    if "nc" in _CACHED:
        return _CACHED["nc"]
    nc = bacc.Bacc("TRN2", target_bir_lowering=False, debug=False, num_devices=N_CORES)
    io = {
        "xq_t": nc.dram_tensor("xq_t", [FQ, S], F32R, kind="ExternalInput").ap(),
        "xk_t": nc.dram_tensor("xk_t", [FKV, S], F32R, kind="ExternalInput").ap(),
        "xv_t": nc.dram_tensor("xv_t", [FKV, S], F32R, kind="ExternalInput").ap(),
        "wq": nc.dram_tensor("wq", [FQ, DH], F32R, kind="ExternalInput").ap(),
        "wk": nc.dram_tensor("wk", [FKV, DH], F32R, kind="ExternalInput").ap(),
        "wv": nc.dram_tensor("wv", [FKV, DH], F32R, kind="ExternalInput").ap(),
        "wo": nc.dram_tensor("wo", [DH, FQ], F32R, kind="ExternalInput").ap(),
        "bq": nc.dram_tensor("bq", [DH], F32, kind="ExternalInput").ap(),
        "bk": nc.dram_tensor("bk", [DH], F32, kind="ExternalInput").ap(),
        "bv": nc.dram_tensor("bv", [DH], F32, kind="ExternalInput").ap(),
        "out_t": nc.dram_tensor("out_t", [FQ, S], F32, kind="ExternalOutput").ap(),
    }
    with tile.TileContext(nc) as tc:
        _emit(nc, tc, io)
    nc.compile()
    _CACHED["nc"] = nc
    return nc


def _round_f32r(a):
    """Round fp32 to the fp32r grid (11 mantissa bits) like the on-chip
    converters do, so the PE sees pre-rounded operands."""
    u = np.ascontiguousarray(a, np.float32).view(np.uint32).astype(np.uint64)
    r = ((u + 0x800) & 0xFFFFF000).astype(np.uint32)
    return r.view(np.float32).reshape(np.shape(a))


def make_in_maps(inputs):
    """Shard full inputs into per-core input maps (host side)."""
    q = _round_f32r(inputs["query"])
    k = _round_f32r(inputs["key"])
    v = _round_f32r(inputs["value"])
    wq_r = _round_f32r(inputs["Wq"])
    wk_r = _round_f32r(inputs["Wk"])
    wv_r = _round_f32r(inputs["Wv"])
    wo_r = _round_f32r(inputs["Wo"])
    in_maps = []
    for c in range(N_CORES):
        b, hg = divmod(c, 2)
        sl = slice(hg * DH, (hg + 1) * DH)
        in_maps.append(
            {
                "xq_t": np.ascontiguousarray(q[b].T),
                "xk_t": np.ascontiguousarray(k[b].T),
                "xv_t": np.ascontiguousarray(v[b].T),
                "wq": np.ascontiguousarray(wq_r[:, sl]),
                "wk": np.ascontiguousarray(wk_r[:, sl]),
                "wv": np.ascontiguousarray(wv_r[:, sl]),
                "wo": np.ascontiguousarray(wo_r[sl, :]),
                "bq": np.ascontiguousarray(np.asarray(inputs["bq"], np.float32)[sl]),
                "bk": np.ascontiguousarray(np.asarray(inputs["bk"], np.float32)[sl]),
                "bv": np.ascontiguousarray(np.asarray(inputs["bv"], np.float32)[sl]),
            }
        )
    return in_maps


def combine(results, bo):
    """Host-side unshard: sum head-group partials, transpose, add bo."""
    out = np.empty((B, S, FQ), np.float32)
    for b in range(B):
        out[b] = (
            results[2 * b]["out_t"].T + results[2 * b + 1]["out_t"].T
        ) + np.asarray(bo, np.float32)
    return out


def run_sharded(inputs, trace=False):
    nc = _build()
    in_maps = make_in_maps(inputs)
    bkr = run_bass_kernel_spmd(nc, in_maps, list(range(N_CORES)), trace=trace)
    return combine(bkr.results, inputs["bo"]), bkr


def kernel(**inputs) -> np.ndarray:
    out, _ = run_sharded(inputs)
    return out



# revision 3
# speedup vs baseline: 1.1493x; 1.1493x over previous
"""Trainium2 Bass kernel: multi-head cross-attention (B=4, Sq=Skv=2048,
query_dim=1024, kv_dim=768, 16 heads x 64).

Sharding: 8 cores = data-parallel over batch (4) x tensor-parallel over
heads (2 groups of 8 heads).

Per-core pipeline (head h in 0..8, d=64):
  Q = (xq_bf16 @ Wq_bf16) + bq      -> qt fp8  [128=2h x 64d, 2048+256pad]
  K = (xk_bf16 @ Wk_bf16)           -> kt8 fp8 [128, 2 slots, 2048] (slot1=0)
      (bk dropped: adds a per-query constant to logits -> softmax-invariant)
  V = (xv_bf16 @ Wv_bf16)           -> vt bf16 [128 k, 8h, 64] per k-block
  S = K~.T @ Q via fp8 DoubleRow (zero second slot) -> st psum [128 k, 1024 q]
  P = exp(S/8): ACT exact exp -> bf16, or DVE/Pool Schraudolph int16 trick
  at[q, d] (+ Z via ones-matmul) = P.T @ V   (P bf16 stationary, full PE rate)
  E = at * (1/Z) -> bf16, PE-transpose -> et [128 dh, 2048 q]
  out_t = Wo.T @ E                  -> [1024, 2048] bf16 partial
Host: out[b] = out_t(hg0).T + out_t(hg1).T + (bv @ Wo + bo)
      (bv folds through: softmax weights sum to 1).
"""

from contextlib import ExitStack

import numpy as np
import ml_dtypes

import concourse.bacc as bacc
import concourse.mybir as mybir
import concourse.tile as tile
from concourse.bass_utils import run_bass_kernel_spmd

F32 = mybir.dt.float32
BF16 = mybir.dt.bfloat16
F8 = mybir.dt.float8e4
I16 = mybir.dt.int16
AF = mybir.ActivationFunctionType
DR = mybir.MatmulPerfMode.DoubleRow
NBF = ml_dtypes.bfloat16

B = 4
S = 2048
FQ = 1024
FKV = 768
DH = 512  # per-core hidden (8 heads x 64)
NH = 8
D = 64
N_CORES = 8

KC_Q = FQ // 128  # 8
KC_KV = FKV // 128  # 6
MT = DH // 128  # 4
KT = S // 128  # 16

SCALE = 0.125  # 1/sqrt(64)
LN2 = float(np.log(2.0))
SCH_K = SCALE * 128.0 / LN2  # folds the 1/8 logit scale
SCH_B = 127.0 * 128.0
SCH_D = -10.0  # tuned shift

# exp unit = (h, kt, half): engine routing. "A"=ACT exact, "D"=DVE, "P"=Pool.
def exp_engine(h, kt, half):
    u = (kt * 2 + half) % 16
    if u in (5, 11):  # 2/16 -> DVE
        return "D"
    if u in (15,):  # 1/16 -> Pool
        return "P"
    return "A"


def _emit_projections(nc, tc, io, qt, kt8, vt, bq_sb):
    with ExitStack() as st8:
        projp = st8.enter_context(tc.tile_pool(name="proj", bufs=1))
        wq_sb = [projp.tile([128, DH], BF16, tag=f"wq{i}", name=f"wq{i}") for i in range(KC_Q)]
        wk_sb = [projp.tile([128, DH], BF16, tag=f"wk{i}", name=f"wk{i}") for i in range(KC_KV)]
        wv_sb = [projp.tile([128, DH], BF16, tag=f"wv{i}", name=f"wv{i}") for i in range(KC_KV)]
        xq_sb = [projp.tile([128, S], BF16, tag=f"xq{i}", name=f"xq{i}") for i in range(KC_Q)]
        xk_sb = [projp.tile([128, S], BF16, tag=f"xk{i}", name=f"xk{i}") for i in range(KC_KV)]
        for i in range(KC_Q):
            nc.sync.dma_start(out=wq_sb[i], in_=io["wq"][i * 128 : (i + 1) * 128, :])
            nc.sync.dma_start(out=xq_sb[i], in_=io["xq_t"][i * 128 : (i + 1) * 128, :])
        for i in range(KC_KV):
            nc.sync.dma_start(out=wk_sb[i], in_=io["wk"][i * 128 : (i + 1) * 128, :])
            nc.sync.dma_start(out=xk_sb[i], in_=io["xk_t"][i * 128 : (i + 1) * 128, :])
            nc.sync.dma_start(out=wv_sb[i], in_=io["wv"][i * 128 : (i + 1) * 128, :])

        # Q and K projections: out [128 dh, 512 q]-chunks; Q += bias -> fp8,
        # K -> fp8 (no bias).
        with tc.tile_pool(name="qkps", bufs=4, space="PSUM") as pps:
            for m in range(MT):
                for qh in range(4):
                    qsl = slice(qh * 512, (qh + 1) * 512)
                    pq = pps.tile([128, 512], F32, tag="pp", name="pq")
                    for kc in range(KC_Q):
                        nc.tensor.matmul(
                            pq,
                            wq_sb[kc][:, m * 128 : (m + 1) * 128],
                            xq_sb[kc][:, qsl],
                            start=(kc == 0),
                            stop=(kc == KC_Q - 1),
                        )
                    nc.vector.tensor_scalar_add(
                        qt[m][:, qsl], pq, bq_sb[:, m : m + 1]
                    )
                    pk = pps.tile([128, 512], F32, tag="pp", name="pk")
                    for kc in range(KC_KV):
                        nc.tensor.matmul(
                            pk,
                            wk_sb[kc][:, m * 128 : (m + 1) * 128],
                            xk_sb[kc][:, qsl],
                            start=(kc == 0),
                            stop=(kc == KC_KV - 1),
                        )
                    nc.gpsimd.tensor_copy(kt8[m][:, 0, qsl], pk)

        # V projection: stationary = xv chunk, moving = wv.
        xv_sb = [projp.tile([128, S], BF16, tag=f"xv{i}", name=f"xv{i}") for i in range(KC_KV)]
        for i in range(KC_KV):
            nc.sync.dma_start(out=xv_sb[i], in_=io["xv_t"][i * 128 : (i + 1) * 128, :])
        with tc.tile_pool(name="vps", bufs=8, space="PSUM") as ppsv:
            for vh in range(2):
                psv = [
                    ppsv.tile([128, DH], F32, tag="pv", name=f"pv{i}") for i in range(8)
                ]
                for kc in range(KC_KV):
                    for ki in range(8):
                        nc.tensor.matmul(
                            psv[ki],
                            xv_sb[kc][:, vh * 1024 + ki * 128 : vh * 1024 + (ki + 1) * 128],
                            wv_sb[kc],
                            start=(kc == 0),
                            stop=(kc == KC_KV - 1),
                        )
                for ki in range(8):
                    nc.gpsimd.tensor_copy(
                        vt[vh * 8 + ki],
                        psv[ki].rearrange("p (h d) -> p h d", h=NH),
                    )


def _emit(nc, tc, io):
    with ExitStack() as stk:
        persist = stk.enter_context(tc.tile_pool(name="persist", bufs=1))

        bq_sb = persist.tile([128, MT], F32, tag="bq")
        nc.sync.dma_start(out=bq_sb, in_=io["bq"].rearrange("(m p) -> p m", p=128))
        ident = persist.tile([128, 128], BF16, tag="ident")
        nc.sync.dma_start(out=ident, in_=io["ident"])
        ones = persist.tile([128, 1], BF16, tag="ones")
        nc.vector.memset(ones, 1.0)

        # Q tiles fp8 with 256-col zero pad for the DR sliding window.
        qt = [persist.tile([128, S + 256], F8, tag=f"qt{i}", name=f"qt{i}") for i in range(MT)]
        # K~ tiles fp8: [128, 2 slots, 2048]; slot 1 stays zero.
        kt8 = [persist.tile([128, 2, S], F8, tag=f"kt{i}", name=f"kt{i}") for i in range(MT)]
        for i in range(MT):
            nc.gpsimd.memset(qt[i][:, S : S + 256], 0.0)
            nc.gpsimd.memset(kt8[i][:, 1, :], 0.0)
        vt = [
            persist.tile([128, NH, D], BF16, tag=f"vt{i}", name=f"vt{i}")
            for i in range(KT)
        ]
        et = [persist.tile([128, S], BF16, tag=f"et{i}", name=f"et{i}") for i in range(MT)]
        wo_sb = [persist.tile([128, FQ], BF16, tag=f"wo{i}", name=f"wo{i}") for i in range(MT)]

        _emit_projections(nc, tc, io, qt, kt8, vt, bq_sb)
        for i in range(MT):
            nc.sync.dma_start(out=wo_sb[i], in_=io["wo"][i * 128 : (i + 1) * 128, :])

        # ---------------- attention ----------------
        attnp = stk.enter_context(tc.tile_pool(name="attn", bufs=1))
        with tc.tile_pool(name="stp", bufs=2, space="PSUM") as stp, tc.tile_pool(
            name="atp", bufs=1, space="PSUM"
        ) as atp, tc.tile_pool(name="zp", bufs=1, space="PSUM") as zpp, tc.tile_pool(
            name="etp", bufs=1, space="PSUM"
        ) as etpp:
            z_all = zpp.tile([128, 2, 16], F32, tag="z", name="z")

            def emit_qk_exp(h, kt, P):
                """QK for (h, kt) into st psum, exp into P[:, half] slices."""
                ht, hr = divmod(h, 2)
                psl = slice(hr * 64, (hr + 1) * 64)
                lhs = kt8[ht][psl, :, kt * 128 : (kt + 1) * 128]
                Pb = P.bitcast(BF16)
                for half in range(2):
                    st_ = stp.tile([128, 1024], F32, tag="st", name="st")
                    for qc in range(4):
                        n0 = half * 1024 + qc * 256
                        rhs = qt[ht][psl, n0 : n0 + 512].rearrange(
                            "p (i n) -> p i n", i=2
                        )
                        nc.tensor.matmul(
                            st_[:, qc * 256 : (qc + 1) * 256],
                            lhs,
                            rhs,
                            start=True,
                            stop=True,
                            perf_mode=DR,
                        )
                    hsl = slice(half * 1024, (half + 1) * 1024)
                    eng = exp_engine(h, kt, half)
                    if eng == "A":
                        nc.scalar.activation(Pb[:, hsl], st_, AF.Exp, scale=SCALE)
                    elif eng == "D":
                        nc.vector.tensor_scalar(
                            out=P[:, hsl],
                            in0=st_,
                            scalar1=float(SCH_K),
                            scalar2=float(SCH_B + SCH_D),
                            op0=mybir.AluOpType.mult,
                            op1=mybir.AluOpType.add,
                        )
                    else:
                        nc.gpsimd.tensor_scalar(
                            out=P[:, hsl],
                            in0=st_,
                            scalar1=float(SCH_K),
                            scalar2=float(SCH_B + SCH_D),
                            op0=mybir.AluOpType.mult,
                            op1=mybir.AluOpType.add,
                        )

            def emit_pv(h, kt, P, at):
                Pb = P.bitcast(BF16)
                zsl = z_all[:, h % 2, :]
                for qb in range(16):
                    nc.tensor.matmul(
                        at[:, qb, :],
                        Pb[:, qb * 128 : (qb + 1) * 128],
                        vt[kt][:, h, :],
                        start=(kt == 0),
                        stop=(kt == KT - 1),
                    )
                    nc.tensor.matmul(
                        zsl[:, qb : qb + 1],
                        Pb[:, qb * 128 : (qb + 1) * 128],
                        ones,
                        start=(kt == 0),
                        stop=(kt == KT - 1),
                    )

            def emit_norm(h, at):
                """1/Z, multiply, PE-transpose into et."""
                ht, hr = divmod(h, 2)
                zr = attnp.tile([128, 16], F32, tag="zr", bufs=2, name="zr")
                nc.vector.reciprocal(zr, z_all[:, h % 2, :])
                eh = attnp.tile([128, 16, D], BF16, tag="eh", bufs=2, name="eh")
                nc.vector.tensor_tensor(
                    out=eh,
                    in0=at,
                    in1=zr.unsqueeze(2).broadcast_to([128, 16, D]),
                    op=mybir.AluOpType.mult,
                )
                for qq in range(4):
                    etps = etpp.tile([64, 512], BF16, tag="etps", name="etps")
                    for j in range(4):
                        qb = qq * 4 + j
                        nc.tensor.matmul(
                            etps[:, j * 128 : (j + 1) * 128],
                            eh[:, qb, :],
                            ident,
                            is_transpose=True,
                        )
                    nc.vector.tensor_copy(
                        et[ht][hr * 64 : (hr + 1) * 64, qq * 512 : (qq + 1) * 512],
                        etps,
                    )

            # software pipeline: PV trails QK/exp by one kt
            P_tiles = {}
            at_tiles = {}
            for h in range(NH):
                at_tiles[h] = atp.tile([128, 16, D], F32, tag="at", name=f"at{h}")
                for kt in range(KT):
                    P = attnp.tile([128, S], I16, tag="P", bufs=3, name="P")
                    P_tiles[(h, kt)] = P
                    emit_qk_exp(h, kt, P)
                    if kt > 0:
                        emit_pv(h, kt - 1, P_tiles.pop((h, kt - 1)), at_tiles[h])
                emit_pv(h, KT - 1, P_tiles.pop((h, KT - 1)), at_tiles[h])
                emit_norm(h, at_tiles.pop(h))

        # ---------------- output projection ----------------
        with tc.tile_pool(name="ops", bufs=2, space="PSUM") as pps_o, tc.tile_pool(
            name="osb", bufs=2
        ) as osbp:
            for ot in range(FQ // 128):
                po = pps_o.tile([128, S], F32, tag="po", name="po")
                for kc in range(MT):
                    lhs = wo_sb[kc][:, ot * 128 : (ot + 1) * 128]
                    for qc in range(4):
                        sl = slice(qc * 512, (qc + 1) * 512)
                        nc.tensor.matmul(
                            po[:, sl],
                            lhs,
                            et[kc][:, sl],
                            start=(kc == 0),
                            stop=(kc == MT - 1),
                        )
                ob = osbp.tile([128, S], BF16, tag="ob", name="ob")
                nc.scalar.copy(ob, po)
                nc.sync.dma_start(out=io["out_t"][ot * 128 : (ot + 1) * 128, :], in_=ob)


_CACHED = {}


def _build():
    if "nc" in _CACHED:
        return _CACHED["nc"]
    nc = bacc.Bacc("TRN2", target_bir_lowering=False, debug=False, num_devices=N_CORES)
    io = {
        "xq_t": nc.dram_tensor("xq_t", [FQ, S], BF16, kind="ExternalInput").ap(),
        "xk_t": nc.dram_tensor("xk_t", [FKV, S], BF16, kind="ExternalInput").ap(),
        "xv_t": nc.dram_tensor("xv_t", [FKV, S], BF16, kind="ExternalInput").ap(),
        "wq": nc.dram_tensor("wq", [FQ, DH], BF16, kind="ExternalInput").ap(),
        "wk": nc.dram_tensor("wk", [FKV, DH], BF16, kind="ExternalInput").ap(),
        "wv": nc.dram_tensor("wv", [FKV, DH], BF16, kind="ExternalInput").ap(),
        "wo": nc.dram_tensor("wo", [DH, FQ], BF16, kind="ExternalInput").ap(),
        "bq": nc.dram_tensor("bq", [DH], F32, kind="ExternalInput").ap(),
        "ident": nc.dram_tensor("ident", [128, 128], BF16, kind="ExternalInput").ap(),
        "out_t": nc.dram_tensor("out_t", [FQ, S], BF16, kind="ExternalOutput").ap(),
    }
    with tile.TileContext(nc) as tc:
        _emit(nc, tc, io)
    nc.compile()
    _CACHED["nc"] = nc
    return nc


def make_in_maps(inputs):
    """Shard full inputs into per-core input maps (host side)."""
    q = np.asarray(inputs["query"], np.float32)
    k = np.asarray(inputs["key"], np.float32)
    v = np.asarray(inputs["value"], np.float32)
    Wq = np.asarray(inputs["Wq"], np.float32)
    Wk = np.asarray(inputs["Wk"], np.float32)
    Wv = np.asarray(inputs["Wv"], np.float32)
    Wo = np.asarray(inputs["Wo"], np.float32)
    bq = np.asarray(inputs["bq"], np.float32)
    ident = np.eye(128, dtype=np.float32).astype(NBF)
    xq = [np.ascontiguousarray(q[b].T).astype(NBF) for b in range(B)]
    xk = [np.ascontiguousarray(k[b].T).astype(NBF) for b in range(B)]
    xv = [np.ascontiguousarray(v[b].T).astype(NBF) for b in range(B)]
    in_maps = []
    for c in range(N_CORES):
        b, hg = divmod(c, 2)
        sl = slice(hg * DH, (hg + 1) * DH)
        in_maps.append(
            {
                "xq_t": xq[b],
                "xk_t": xk[b],
                "xv_t": xv[b],
                "wq": np.ascontiguousarray(Wq[:, sl]).astype(NBF),
                "wk": np.ascontiguousarray(Wk[:, sl]).astype(NBF),
                "wv": np.ascontiguousarray(Wv[:, sl]).astype(NBF),
                "wo": np.ascontiguousarray(Wo[sl, :]).astype(NBF),
                "bq": np.ascontiguousarray(bq[sl]),
                "ident": ident,
            }
        )
    return in_maps


def combine(results, inputs):
    """Host-side unshard: sum head-group partials, transpose, add folded bias."""
    Wo = np.asarray(inputs["Wo"], np.float32)
    bias = (
        np.asarray(inputs["bv"], np.float32) @ Wo + np.asarray(inputs["bo"], np.float32)
    ).astype(np.float32)
    out = np.empty((B, S, FQ), np.float32)
    for b in range(B):
        out[b] = (
            results[2 * b]["out_t"].astype(np.float32).T
            + results[2 * b + 1]["out_t"].astype(np.float32).T
        ) + bias
    return out


def run_sharded(inputs, trace=False):
    nc = _build()
    in_maps = make_in_maps(inputs)
    bkr = run_bass_kernel_spmd(nc, in_maps, list(range(N_CORES)), trace=trace)
    return combine(bkr.results, inputs), bkr


def kernel(**inputs) -> np.ndarray:
    out, _ = run_sharded(inputs)
    return out


# revision 6
# speedup vs baseline: 1.2027x; 1.0465x over previous
"""Trainium2 Bass kernel: multi-head cross-attention (B=4, Sq=Skv=2048,
query_dim=1024, kv_dim=768, 16 heads x 64).

Sharding: 8 cores = data-parallel over batch (4) x tensor-parallel over
heads (2 groups of 8 heads).

Per-core pipeline (head h in 0..8, d=64):
  Q = (xq_bf16 @ Wq_bf16) + bq      -> qt fp8  [128=2h x 64d, 2048+256pad]
  K = (xk_bf16 @ Wk_bf16)           -> kt8 fp8 [128, 2 slots, 2048] (slot1=0)
      (bk dropped: adds a per-query constant to logits -> softmax-invariant)
  V = (xv_bf16 @ Wv_bf16)           -> vt bf16 [128 k, 8h, 64] per k-block
  S = K~.T @ Q via fp8 DoubleRow (zero second slot) -> st psum [128 k, 1024 q]
  P = exp(S/8): ACT exact exp -> bf16, or DVE/Pool Schraudolph int16 trick
  at[q, d] (+ Z via ones-matmul) = P.T @ V   (P bf16 stationary, full PE rate)
  E = at * (1/Z) -> bf16, PE-transpose -> et [128 dh, 2048 q]
  out_t = Wo.T @ E                  -> [1024, 2048] bf16 partial
Host: out[b] = out_t(hg0).T + out_t(hg1).T + (bv @ Wo + bo)
      (bv folds through: softmax weights sum to 1).
"""

from contextlib import ExitStack

import numpy as np
import ml_dtypes

import concourse.bacc as bacc
import concourse.mybir as mybir
import concourse.tile as tile
from concourse.bass_utils import run_bass_kernel_spmd

F32 = mybir.dt.float32
BF16 = mybir.dt.bfloat16
F8 = mybir.dt.float8e4
I16 = mybir.dt.int16
AF = mybir.ActivationFunctionType
DR = mybir.MatmulPerfMode.DoubleRow
NBF = ml_dtypes.bfloat16

B = 4
S = 2048
FQ = 1024
FKV = 768
DH = 512  # per-core hidden (8 heads x 64)
NH = 8
D = 64
N_CORES = 8

KC_Q = FQ // 128  # 8
KC_KV = FKV // 128  # 6
MT = DH // 128  # 4
KT = S // 128  # 16

SCALE = 0.125  # 1/sqrt(64)
LN2 = float(np.log(2.0))
SCH_K = SCALE * 128.0 / LN2  # folds the 1/8 logit scale
SCH_B = 127.0 * 128.0
SCH_D = -10.0  # tuned shift

# exp unit = (h, kt, half): engine routing. "A"=ACT exact, "D"=DVE, "P"=Pool.
# 8/16 ACT, 5/16 DVE-Schraudolph, 3/16 Pool-Schraudolph.
_EXP_D = frozenset((0, 3, 6, 9, 12))
_EXP_P = frozenset((2, 7, 13))


def exp_engine(h, kt, half):
    u = (kt * 2 + half) % 16
    if u in _EXP_D:
        return "D"
    if u in _EXP_P:
        return "P"
    return "A"


def _emit_projections(nc, tc, io, qt, kt8, vt, bq_sb):
    with ExitStack() as st8:
        projp = st8.enter_context(tc.tile_pool(name="proj", bufs=1))
        wq_sb = [projp.tile([128, DH], BF16, tag=f"wq{i}", name=f"wq{i}") for i in range(KC_Q)]
        wk_sb = [projp.tile([128, DH], BF16, tag=f"wk{i}", name=f"wk{i}") for i in range(KC_KV)]
        wv_sb = [projp.tile([128, DH], BF16, tag=f"wv{i}", name=f"wv{i}") for i in range(KC_KV)]
        xq_sb = [projp.tile([128, S], BF16, tag=f"xq{i}", name=f"xq{i}") for i in range(KC_Q)]
        xk_sb = [projp.tile([128, S], BF16, tag=f"xk{i}", name=f"xk{i}") for i in range(KC_KV)]
        xv_sb = [projp.tile([128, S], BF16, tag=f"xv{i}", name=f"xv{i}") for i in range(KC_KV)]
        # DMA order: Q weights, then per-q-slice activation chunks so the
        # first projection matmuls can start after ~14 small DMAs.
        for i in range(KC_Q):
            nc.sync.dma_start(out=wq_sb[i], in_=io["wq"][i * 128 : (i + 1) * 128, :])
        for i in range(KC_KV):
            nc.sync.dma_start(out=wk_sb[i], in_=io["wk"][i * 128 : (i + 1) * 128, :])
        for qh in range(4):
            qsl = slice(qh * 512, (qh + 1) * 512)
            for i in range(KC_Q):
                nc.sync.dma_start(
                    out=xq_sb[i][:, qsl], in_=io["xq_t"][i * 128 : (i + 1) * 128, qsl]
                )
            for i in range(KC_KV):
                nc.sync.dma_start(
                    out=xk_sb[i][:, qsl], in_=io["xk_t"][i * 128 : (i + 1) * 128, qsl]
                )
            if qh == 0:
                for i in range(KC_KV):
                    nc.sync.dma_start(
                        out=wv_sb[i], in_=io["wv"][i * 128 : (i + 1) * 128, :]
                    )
            else:
                for i in range(KC_KV):
                    nc.sync.dma_start(
                        out=xv_sb[i][:, slice((qh - 1) * 512, qh * 512)],
                        in_=io["xv_t"][i * 128 : (i + 1) * 128, (qh - 1) * 512 : qh * 512],
                    )
        for i in range(KC_KV):
            nc.sync.dma_start(
                out=xv_sb[i][:, 1536:2048], in_=io["xv_t"][i * 128 : (i + 1) * 128, 1536:2048]
            )

        # Q and K projections: out [128 dh, 512 q]-chunks; Q += bias -> fp8,
        # K -> fp8 (no bias).  q-slice outer to match DMA arrival order.
        with tc.tile_pool(name="qkps", bufs=4, space="PSUM") as pps:
            for qh in range(4):
                qsl = slice(qh * 512, (qh + 1) * 512)
                for m in range(MT):
                    pq = pps.tile([128, 512], F32, tag="pp", name="pq")
                    for kc in range(KC_Q):
                        nc.tensor.matmul(
                            pq,
                            wq_sb[kc][:, m * 128 : (m + 1) * 128],
                            xq_sb[kc][:, qsl],
                            start=(kc == 0),
                            stop=(kc == KC_Q - 1),
                        )
                    nc.vector.tensor_scalar_add(
                        qt[m][:, qsl], pq, bq_sb[:, m : m + 1]
                    )
                    pk = pps.tile([128, 512], F32, tag="pp", name="pk")
                    for kc in range(KC_KV):
                        nc.tensor.matmul(
                            pk,
                            wk_sb[kc][:, m * 128 : (m + 1) * 128],
                            xk_sb[kc][:, qsl],
                            start=(kc == 0),
                            stop=(kc == KC_KV - 1),
                        )
                    nc.gpsimd.tensor_copy(kt8[m][:, 0, qsl], pk)

        # V projection: stationary = xv chunk, moving = wv.
        with tc.tile_pool(name="vps", bufs=8, space="PSUM") as ppsv:
            for vh in range(2):
                psv = [
                    ppsv.tile([128, DH], F32, tag="pv", name=f"pv{i}") for i in range(8)
                ]
                for kc in range(KC_KV):
                    for ki in range(8):
                        nc.tensor.matmul(
                            psv[ki],
                            xv_sb[kc][:, vh * 1024 + ki * 128 : vh * 1024 + (ki + 1) * 128],
                            wv_sb[kc],
                            start=(kc == 0),
                            stop=(kc == KC_KV - 1),
                        )
                for ki in range(8):
                    nc.gpsimd.tensor_copy(
                        vt[vh * 8 + ki],
                        psv[ki].rearrange("p (h d) -> p h d", h=NH),
                    )


def _emit(nc, tc, io):
    with ExitStack() as stk:
        persist = stk.enter_context(tc.tile_pool(name="persist", bufs=1))

        bq_sb = persist.tile([128, MT], F32, tag="bq")
        nc.sync.dma_start(out=bq_sb, in_=io["bq"].rearrange("(m p) -> p m", p=128))
        ident = persist.tile([128, 128], BF16, tag="ident")
        nc.sync.dma_start(out=ident, in_=io["ident"])
        ones = persist.tile([128, 1], BF16, tag="ones")
        nc.vector.memset(ones, 1.0)

        # Q tiles fp8 with 256-col zero pad for the DR sliding window.
        qt = [persist.tile([128, S + 256], F8, tag=f"qt{i}", name=f"qt{i}") for i in range(MT)]
        # K~ tiles fp8: [128, 2 slots, 2048]; slot 1 stays zero.
        kt8 = [persist.tile([128, 2, S], F8, tag=f"kt{i}", name=f"kt{i}") for i in range(MT)]
        for i in range(MT):
            nc.gpsimd.memset(qt[i][:, S : S + 256], 0.0)
            nc.gpsimd.memset(kt8[i][:, 1, :], 0.0)
        vt = [
            persist.tile([128, NH, D], BF16, tag=f"vt{i}", name=f"vt{i}")
            for i in range(KT)
        ]
        et = [persist.tile([128, S], BF16, tag=f"et{i}", name=f"et{i}") for i in range(MT)]
        wo_sb = [persist.tile([128, FQ], BF16, tag=f"wo{i}", name=f"wo{i}") for i in range(MT)]

        _emit_projections(nc, tc, io, qt, kt8, vt, bq_sb)
        for i in range(MT):
            nc.sync.dma_start(out=wo_sb[i], in_=io["wo"][i * 128 : (i + 1) * 128, :])

        # ---------------- attention ----------------
        attnp = stk.enter_context(tc.tile_pool(name="attn", bufs=1))
        with tc.tile_pool(name="stp", bufs=2, space="PSUM") as stp, tc.tile_pool(
            name="atp", bufs=1, space="PSUM"
        ) as atp, tc.tile_pool(name="zp", bufs=1, space="PSUM") as zpp, tc.tile_pool(
            name="etp", bufs=1, space="PSUM"
        ) as etpp:
            z_all = zpp.tile([128, 2, 16], F32, tag="z", name="z")

            def emit_qk_exp(h, kt, P):
                """QK for (h, kt) into st psum, exp into P[:, half] slices."""
                ht, hr = divmod(h, 2)
                psl = slice(hr * 64, (hr + 1) * 64)
                lhs = kt8[ht][psl, :, kt * 128 : (kt + 1) * 128]
                Pb = P.bitcast(BF16)
                for half in range(2):
                    st_ = stp.tile([128, 1024], F32, tag="st", name="st")
                    for qc in range(4):
                        n0 = half * 1024 + qc * 256
                        rhs = qt[ht][psl, n0 : n0 + 512].rearrange(
                            "p (i n) -> p i n", i=2
                        )
                        nc.tensor.matmul(
                            st_[:, qc * 256 : (qc + 1) * 256],
                            lhs,
                            rhs,
                            start=True,
                            stop=True,
                            perf_mode=DR,
                        )
                    hsl = slice(half * 1024, (half + 1) * 1024)
                    eng = exp_engine(h, kt, half)
                    if eng == "A":
                        nc.scalar.activation(Pb[:, hsl], st_, AF.Exp, scale=SCALE)
                    elif eng == "D":
                        nc.vector.tensor_scalar(
                            out=P[:, hsl],
                            in0=st_,
                            scalar1=float(SCH_K),
                            scalar2=float(SCH_B + SCH_D),
                            op0=mybir.AluOpType.mult,
                            op1=mybir.AluOpType.add,
                        )
                    else:
                        nc.gpsimd.tensor_scalar(
                            out=P[:, hsl],
                            in0=st_,
                            scalar1=float(SCH_K),
                            scalar2=float(SCH_B + SCH_D),
                            op0=mybir.AluOpType.mult,
                            op1=mybir.AluOpType.add,
                        )

            def emit_pv(h, kt, P, at):
                Pb = P.bitcast(BF16)
                zsl = z_all[:, h % 2, :]
                for qb in range(16):
                    nc.tensor.matmul(
                        at[:, qb, :],
                        Pb[:, qb * 128 : (qb + 1) * 128],
                        vt[kt][:, h, :],
                        start=(kt == 0),
                        stop=(kt == KT - 1),
                    )
                    nc.tensor.matmul(
                        zsl[:, qb : qb + 1],
                        Pb[:, qb * 128 : (qb + 1) * 128],
                        ones,
                        start=(kt == 0),
                        stop=(kt == KT - 1),
                    )

            eh_tiles = {}

            def emit_norm_a(h, at):
                """1/Z and the normalize-multiply (DVE)."""
                zr = attnp.tile([128, 16], F32, tag="zr", bufs=2, name="zr")
                nc.vector.reciprocal(zr, z_all[:, h % 2, :])
                eh = attnp.tile([128, 16, D], BF16, tag="eh", bufs=2, name="eh")
                nc.vector.tensor_tensor(
                    out=eh,
                    in0=at,
                    in1=zr.unsqueeze(2).broadcast_to([128, 16, D]),
                    op=mybir.AluOpType.mult,
                )
                eh_tiles[h] = eh

            def emit_norm_b(h):
                """PE-transpose eh into et."""
                ht, hr = divmod(h, 2)
                eh = eh_tiles.pop(h)
                for qq in range(4):
                    etps = etpp.tile([64, 512], BF16, tag="etps", name="etps")
                    for j in range(4):
                        qb = qq * 4 + j
                        nc.tensor.matmul(
                            etps[:, j * 128 : (j + 1) * 128],
                            eh[:, qb, :],
                            ident,
                            is_transpose=True,
                        )
                    nc.vector.tensor_copy(
                        et[ht][hr * 64 : (hr + 1) * 64, qq * 512 : (qq + 1) * 512],
                        etps,
                    )

            # software pipeline: PV trails QK/exp by one kt; norm of the
            # previous head is emitted after the next head's pipeline starts
            # so the PE never waits on the DVE norm chain.
            P_tiles = {}
            at_tiles = {}
            for h in range(NH):
                at_tiles[h] = atp.tile([128, 16, D], F32, tag="at", name=f"at{h}")
                for kt in range(KT):
                    P = attnp.tile([128, S], I16, tag="P", bufs=4, name="P")
                    P_tiles[(h, kt)] = P
                    emit_qk_exp(h, kt, P)
                    if kt == 1 and h > 0:
                        emit_norm_a(h - 1, at_tiles.pop(h - 1))
                    if kt == 3 and h > 0:
                        emit_norm_b(h - 1)
                    if kt > 0:
                        emit_pv(h, kt - 1, P_tiles.pop((h, kt - 1)), at_tiles[h])
                emit_pv(h, KT - 1, P_tiles.pop((h, KT - 1)), at_tiles[h])
            emit_norm_a(NH - 1, at_tiles.pop(NH - 1))
            emit_norm_b(NH - 1)

        # ---------------- output projection ----------------
        with tc.tile_pool(name="ops", bufs=2, space="PSUM") as pps_o, tc.tile_pool(
            name="osb", bufs=2
        ) as osbp:
            for ot in range(FQ // 128):
                po = pps_o.tile([128, S], F32, tag="po", name="po")
                for kc in range(MT):
                    lhs = wo_sb[kc][:, ot * 128 : (ot + 1) * 128]
                    for qc in range(4):
                        sl = slice(qc * 512, (qc + 1) * 512)
                        nc.tensor.matmul(
                            po[:, sl],
                            lhs,
                            et[kc][:, sl],
                            start=(kc == 0),
                            stop=(kc == MT - 1),
                        )
                ob = osbp.tile([128, S], BF16, tag="ob", name="ob")
                nc.scalar.copy(ob, po)
                nc.sync.dma_start(out=io["out_t"][ot * 128 : (ot + 1) * 128, :], in_=ob)


_CACHED = {}


def _build():
    if "nc" in _CACHED:
        return _CACHED["nc"]
    nc = bacc.Bacc("TRN2", target_bir_lowering=False, debug=False, num_devices=N_CORES)
    io = {
        "xq_t": nc.dram_tensor("xq_t", [FQ, S], BF16, kind="ExternalInput").ap(),
        "xk_t": nc.dram_tensor("xk_t", [FKV, S], BF16, kind="ExternalInput").ap(),
        "xv_t": nc.dram_tensor("xv_t", [FKV, S], BF16, kind="ExternalInput").ap(),
        "wq": nc.dram_tensor("wq", [FQ, DH], BF16, kind="ExternalInput").ap(),
        "wk": nc.dram_tensor("wk", [FKV, DH], BF16, kind="ExternalInput").ap(),
        "wv": nc.dram_tensor("wv", [FKV, DH], BF16, kind="ExternalInput").ap(),
        "wo": nc.dram_tensor("wo", [DH, FQ], BF16, kind="ExternalInput").ap(),
        "bq": nc.dram_tensor("bq", [DH], F32, kind="ExternalInput").ap(),
        "ident": nc.dram_tensor("ident", [128, 128], BF16, kind="ExternalInput").ap(),
        "out_t": nc.dram_tensor("out_t", [FQ, S], BF16, kind="ExternalOutput").ap(),
    }
    with tile.TileContext(nc) as tc:
        _emit(nc, tc, io)
    nc.compile()
    _CACHED["nc"] = nc
    return nc


def make_in_maps(inputs):
    """Shard full inputs into per-core input maps (host side)."""
    q = np.asarray(inputs["query"], np.float32)
    k = np.asarray(inputs["key"], np.float32)
    v = np.asarray(inputs["value"], np.float32)
    Wq = np.asarray(inputs["Wq"], np.float32)
    Wk = np.asarray(inputs["Wk"], np.float32)
    Wv = np.asarray(inputs["Wv"], np.float32)
    Wo = np.asarray(inputs["Wo"], np.float32)
    bq = np.asarray(inputs["bq"], np.float32)
    ident = np.eye(128, dtype=np.float32).astype(NBF)
    xq = [np.ascontiguousarray(q[b].T).astype(NBF) for b in range(B)]
    xk = [np.ascontiguousarray(k[b].T).astype(NBF) for b in range(B)]
    xv = [np.ascontiguousarray(v[b].T).astype(NBF) for b in range(B)]
    in_maps = []
    for c in range(N_CORES):
        b, hg = divmod(c, 2)
        sl = slice(hg * DH, (hg + 1) * DH)
        in_maps.append(
            {
                "xq_t": xq[b],
                "xk_t": xk[b],
                "xv_t": xv[b],
                "wq": np.ascontiguousarray(Wq[:, sl]).astype(NBF),
                "wk": np.ascontiguousarray(Wk[:, sl]).astype(NBF),
                "wv": np.ascontiguousarray(Wv[:, sl]).astype(NBF),
                "wo": np.ascontiguousarray(Wo[sl, :]).astype(NBF),
                "bq": np.ascontiguousarray(bq[sl]),
                "ident": ident,
            }
        )
    return in_maps


def combine(results, inputs):
    """Host-side unshard: sum head-group partials, transpose, add folded bias."""
    Wo = np.asarray(inputs["Wo"], np.float32)
    bias = (
        np.asarray(inputs["bv"], np.float32) @ Wo + np.asarray(inputs["bo"], np.float32)
    ).astype(np.float32)
    out = np.empty((B, S, FQ), np.float32)
    for b in range(B):
        out[b] = (
            results[2 * b]["out_t"].astype(np.float32).T
            + results[2 * b + 1]["out_t"].astype(np.float32).T
        ) + bias
    return out


def run_sharded(inputs, trace=False):
    nc = _build()
    in_maps = make_in_maps(inputs)
    bkr = run_bass_kernel_spmd(nc, in_maps, list(range(N_CORES)), trace=trace)
    return combine(bkr.results, inputs), bkr


def kernel(**inputs) -> np.ndarray:
    out, _ = run_sharded(inputs)
    return out
